# revision 5
# baseline (speedup 1.0000x reference)
"""Trainium2 Bass kernel for nn_MultiHeadAttention_78134045049371.

Strategy (8 NeuronCores, batch x head hybrid sharding):
  - Cores are split into 2 groups of 4 (group = batch). Core (b, g) owns
    batch b and heads [4g, 4g+4); a group covers all 16 heads of its
    batch, so AllGathers span 4 ranks and the two groups' collectives run
    on disjoint dies.
  - Host feeds per-batch q/k/v TRANSPOSED and pre-tiled ([128, nt, c, 512]
    fp16) plus per-core weight slices, so every matmul contracts over the
    partition axis and every DMA partition line is one contiguous run.
  - The 4 heads per core are processed as 2 head-pairs (hp); each
    (q-block, hp) attention unit is the transposed-scores loop:
    S^T [keys, q] with the two heads in PE row groups (0,0)/(64,0), exp on
    [128, 2*QB] PSUM spans, and a ones column in the V stationary so
    O_aug = [V|1]^T @ exp(S^T) accumulates output + exp-sum rows.
  - PRODUCER-side softmax normalization: after the kt loop, 1/sumexp is
    computed on DVE (partition 64, base-aligned), broadcast down to
    partitions 0-63 with a K=1 matmul (ones stationary), and fused into
    the PSUM->SBUF contribution copy as a tensor_mul. The gathered data
    is already normalized -> no post-gather reciprocal/broadcast chain.
  - Per-(q-block, head-pair) AllGather of the [128, QB] normalized
    contribution; gathered chunks land as fc contraction chunks directly
    (weights host-reordered to chunk order c = hp*4 + g).
  - fc blocks are emitted late enough that their gathers are long done
    even with ~30us cross-core launch skew (no tensor-FIFO blocking).
  - A short burst of dummy matmuls during the initial DMA ramp trips the
    PE HAM activity window so real matmuls run at 2.4 GHz from the start.
  - Output is written transposed fp16 [256, L]; host reassembles/casts.
"""

import sys

for _p in ("/opt/trn_rl_repo", "/root/.axon_site/_ro/trn_rl_repo"):
    if _p not in sys.path:
        sys.path.append(_p)

import numpy as np

import concourse.bass as bass
import concourse.mybir as mybir
import concourse.tile as tile
from concourse import bass_utils
from concourse.vector_clock import ScopedClock

# Problem shape (fixed by the reference)
B, L, D = 2, 2048, 1024
H, DK, DV = 16, 64, 64
NC = 8  # cores
NG = 4  # cores per group (= ranks per AllGather)
HC = H // NG  # heads per core = 4
HL = 2  # heads per head-pair (attention unit)
NHP = HC // HL  # head-pairs per core = 2
TEMP = float(np.sqrt(DK))  # 8.0

NQB = 4  # q-blocks per core (one batch: L rows)
QB = L // NQB  # 512 columns per q-block
KT = 128  # key tile (partition dim of S^T)
NKT = L // KT  # 16 key tiles
DCH = D // 128  # 8 contraction chunks of 128

F16 = mybir.dt.float16
F32 = mybir.dt.float32

MAX_WAITS = 1  # this walrus build encodes at most 1 sem-wait per instruction
N_WARM = 10  # dummy matmuls to trip the PE HAM window during the DMA ramp


def _split_excess_waits(nc):
    """Move excess sem-waits onto NOPs inserted just before the owning
    instruction on the same engine (engine queues are FIFO, so semantics
    are preserved). The walrus build here rejects >1 wait per instruction."""
    for f in nc.m.functions:
        for bb in f.blocks:
            out = []
            changed = False
            for inst in bb.instructions:
                si = inst.sync_info
                waits = list(si.on_wait) if si and si.on_wait else []
                if len(waits) > MAX_WAITS:
                    changed = True
                    k = 0
                    while len(waits) > MAX_WAITS:
                        chunk, waits = waits[:MAX_WAITS], waits[MAX_WAITS:]
                        nop = mybir.InstNoOp(
                            name=f"{inst.name}-wsplit-{k}", ins=[], outs=[]
                        )
                        nop.engine = inst.engine
                        nop.sync_info = mybir.SyncInfo(on_wait=chunk, on_update=[])
                        nc.register_instruction(nop, overwrite=True)
                        out.append(nop)
                        k += 1
                    si.on_wait = waits
                    inst.sync_info = si
                out.append(inst)
            if changed:
                bb.instructions = out


class _TileContext(tile.TileContext):
    """TileContext whose final drain carries its waits on separate NOPs."""

    def _drain_and_barrier(self, tick_clock, wait_clock):
        nc = self.nc
        collector = nc.sync.nop(nofuse=True)
        wait_clock.add_sem_waits(
            collector.ins, ScopedClock({None: tick_clock.global_clock})
        )
        nc.sync.drain()
        nc.all_engine_barrier()
        popped = nc._tile_sem_poison_stack.pop()
        assert popped is self._sem_poison
        nc.clear_and_free_semaphores(list(self.sems.allocated().values()))
        nc.all_engine_barrier()

    def __exit__(self, exc_type, exc_value, traceback):
        super().__exit__(exc_type, exc_value, traceback)
        if exc_type is None:
            _split_excess_waits(self.nc)


def build_kernel():
    nc = bass.Bass(target_bir_lowering=False)

    # Inputs (per core): pre-tiled transposed activations for THIS CORE'S
    # BATCH and per-core weight slices, all fp16. Layout [128, nt, c, n]:
    # partition p holds contraction-row c*128+p, column block nt.
    qT = nc.dram_tensor("qT", [128, NQB, DCH, 512], F16, kind="ExternalInput")
    kT = nc.dram_tensor("kT", [128, NQB, DCH, 512], F16, kind="ExternalInput")
    vT = nc.dram_tensor("vT", [128, NQB, DCH, 512], F16, kind="ExternalInput")
    # [128, c, 256]: columns = this core's 4 heads x 64 (q pre-scaled 1/8)
    wqT = nc.dram_tensor("wqT", [128, DCH, HC * DK], F16, kind="ExternalInput")
    wkT = nc.dram_tensor("wkT", [128, DCH, HC * DK], F16, kind="ExternalInput")
    wvT = nc.dram_tensor("wvT", [128, DCH, HC * DV], F16, kind="ExternalInput")
    # [128, c, 256]: this core's 256 output columns of Wfc/Wg; contraction
    # chunk c = hp*4 + g (matches the gathered contribution layout)
    wfcT = nc.dram_tensor("wfcT", [128, DCH, 2 * 128], F16, kind="ExternalInput")
    wgT = nc.dram_tensor("wgT", [128, DCH, 2 * 128], F16, kind="ExternalInput")

    # Output: this core's 256 output columns for its batch's L rows,
    # stored transposed ([dout, row]) fp16; host reassembles + casts.
    out = nc.dram_tensor("out", [2 * 128, L], F16, kind="ExternalOutput")

    # AllGather buffers: per (q-block, head-pair) NORMALIZED contribution
    # [128, QB] fp16 (two heads x 64 O^T rows) -> gathered [NG*128, QB].
    ag_in = nc.dram_tensor("ag_in", [NQB, NHP, 128, QB], F16)
    ag_out = nc.dram_tensor("ag_out", [NQB, NHP, NG * 128, QB], F16)

    GROUPS = [[0, 1, 2, 3], [4, 5, 6, 7]]

    with _TileContext(nc) as tc:
        with (
            tc.tile_pool(name="persist", bufs=1) as persist,
            tc.tile_pool(name="astream", bufs=4) as astream,
            tc.tile_pool(name="exps", bufs=10) as exps,
            tc.tile_pool(name="small", bufs=3) as small,
            tc.tile_pool(name="fcin", bufs=2) as fcin,
            tc.tile_pool(name="pp_o", bufs=2, space="PSUM") as pp_o,
            tc.tile_pool(name="pp_fc", bufs=2, space="PSUM") as pp_fc,
            tc.tile_pool(name="pp_s", bufs=2, space="PSUM") as pp_s,
        ):
            # ---- resident tiles ----
            qhTs = [
                persist.tile([HL * DK, QB], F16, name=f"qhT{i}")
                for i in range(NQB * NHP)  # index qb*NHP + hp
            ]
            khTs = [
                persist.tile([HL * DK, L], F16, name=f"khT{i}") for i in range(NHP)
            ]
            # vh augmented with a ones column per head: [head][0:64]=vh, [64]=1
            vhs = [
                persist.tile([128, L // 128, HL * (DV + 1)], F16, name=f"vh{i}")
                for i in range(NHP)
            ]
            wq_sb = persist.tile([128, DCH, HC * DK], F16)
            wk_sb = persist.tile([128, DCH, HC * DK], F16)
            wv_sb = persist.tile([128, DCH, HC * DV], F16)
            wfc_sb = persist.tile([128, DCH, 2 * 128], F16)
            wg_sb = persist.tile([128, DCH, 2 * 128], F16)
            # ones stationary (row 64 used for the normalizer broadcast)
            ones_sb = persist.tile([128, DV], F16, name="ones")
            warm_sb = persist.tile([128, 512], F16, name="warm")

            nc.vector.memset(ones_sb[:], 1.0)
            nc.vector.memset(warm_sb[:], 0.0)
            for vh in vhs:
                nc.vector.memset(vh[:, :, DV : DV + 1], 1.0)
                nc.vector.memset(vh[:, :, DV + 1 + DV :], 1.0)

            # ---- early weight DMAs + PE warmup during the input ramp ----
            nc.sync.dma_start(out=wk_sb[:], in_=wkT[:])
            nc.sync.dma_start(out=wq_sb[:], in_=wqT[:])
            for i in range(N_WARM):
                wps = pp_fc.tile([128, 512], F32, tag="fcpsum", name="warmps")
                nc.tensor.matmul(
                    wps[0:64, :],
                    lhsT=warm_sb[:, 0:DV],
                    rhs=warm_sb[:],
                    start=True,
                    stop=True,
                )

            # ---- projections ----
            def proj_kq(src, wsb, dsts, nt):
                # dsts[hp] [128, 512] = sum_c w[c, hp].T @ xT[c] for block nt
                xt = astream.tile([128, DCH, 512], F16, tag="xproj", name="xt")
                nc.sync.dma_start(out=xt[:], in_=src[:, nt])
                for hp in range(NHP):
                    ps = pp_fc.tile([128, 512], F32, tag="fcpsum", name="psq")
                    for c in range(DCH):
                        nc.tensor.matmul(
                            ps[:],
                            lhsT=wsb[:, c, hp * 128 : (hp + 1) * 128],
                            rhs=xt[:, c, :],
                            start=(c == 0),
                            stop=(c == DCH - 1),
                        )
                    nc.vector.tensor_copy(out=dsts[hp][:], in_=ps[:])

            def proj_v(nt):
                # both head-pairs per matmul (N=256); strided copies peel
                # the per-head 64-column slices into the [V|1] slots
                vt = astream.tile([128, DCH, 512], F16, tag="vproj", name="vt")
                nc.sync.dma_start(out=vt[:], in_=vT[:, nt])
                for sub in range(4):
                    loc = nt * 4 + sub
                    ps = pp_fc.tile([128, 512], F32, tag="fcpsum", name="psv")
                    for c in range(DCH):
                        nc.tensor.matmul(
                            ps[:, 0 : NHP * 128],
                            lhsT=vt[:, c, bass.ts(sub, 128)],
                            rhs=wv_sb[:, c, :],
                            start=(c == 0),
                            stop=(c == DCH - 1),
                        )
                    for hp in range(NHP):
                        dst = vhs[hp][:, loc, :].rearrange(
                            "p (h x) -> p h x", h=HL
                        )[:, :, 0:DV]
                        nc.vector.tensor_copy(
                            out=dst,
                            in_=ps[:, hp * 128 : (hp + 1) * 128].rearrange(
                                "p (h x) -> p h x", h=HL
                            ),
                        )

            # ---- attention unit (q-block, head-pair) ----
            # S matmuls for the two heads sit at PE row groups (0,0)/(64,0)
            # and execute concurrently. exp runs on [128, 2*QB] PSUM spans.
            def attention(qb, hp, pre_s=None, pre_o0=()):
                pre_s = pre_s or {}
                opsums = [
                    pp_o.tile([DV + 1, QB], F32, tag="opsum", name=f"ops{h}")
                    for h in range(HL)
                ]
                for kt in range(NKT):
                    for f in pre_s.get(kt, ()):
                        f()
                    sps = pp_s.tile([KT, HL * QB], F32, tag="spsum")
                    for h in range(HL):
                        hp_ = h * DK
                        nc.tensor.matmul(
                            sps[:, h * QB : (h + 1) * QB],
                            lhsT=khTs[hp][hp_ : hp_ + DK, kt * KT : (kt + 1) * KT],
                            rhs=qhTs[qb * NHP + hp][hp_ : hp_ + DK, :],
                            start=True,
                            stop=True,
                        )
                    et = exps.tile([KT, HL * QB], F16, tag="expst")
                    nc.scalar.activation(
                        out=et[:],
                        in_=sps[:],
                        func=mybir.ActivationFunctionType.Exp,
                    )
                    if kt == 0:
                        for f in pre_o0:
                            f()
                    for h in range(HL):
                        nc.tensor.matmul(
                            opsums[h][:],
                            lhsT=vhs[hp][:, kt, h * (DV + 1) : (h + 1) * (DV + 1)],
                            rhs=et[:, h * QB : (h + 1) * QB],
                            start=(kt == 0),
                            stop=(kt == NKT - 1),
                        )
                return opsums

            # ---- producer-side normalization + contribution ship ----
            def norm_ship(qb, hp, opsums):
                # 1/sumexp on partition 64 (base-aligned with the PSUM
                # sumexp row), then a K=1 matmul broadcasts it down to
                # partitions 0-63 where the fused normalize-copy runs.
                rec = small.tile([128, HL * QB], F16, tag="rec", name="rec")
                with nc.allow_low_precision(reason="softmax normalizer in fp16"):
                    for h in range(HL):
                        nc.vector.reciprocal(
                            out=rec[DV : DV + 1, bass.ts(h, QB)],
                            in_=opsums[h][DV : DV + 1, :],
                        )
                rbs = []
                for h in range(HL):
                    rb = pp_fc.tile([DV, QB], F32, tag="fcpsum", name=f"rb{h}")
                    nc.tensor.matmul(
                        rb[:],
                        lhsT=ones_sb[DV : DV + 1, :],
                        rhs=rec[DV : DV + 1, bass.ts(h, QB)],
                        start=True,
                        stop=True,
                    )
                    rb_sb = small.tile([DV, QB], F16, tag="rbsb", name="rb_sb")
                    nc.vector.tensor_copy(out=rb_sb[:], in_=rb[:])
                    rbs.append(rb_sb)
                for h in range(HL):
                    o_sb = small.tile([DV, QB], F16, tag="osb", name="o_sb")
                    nc.vector.tensor_copy(out=o_sb[:], in_=opsums[h][0:DV, :])
                    ct = small.tile([DV, QB], F16, tag="contrib", name="ct")
                    nc.vector.tensor_mul(out=ct[:], in0=o_sb[:], in1=rbs[h][:])
                    nc.gpsimd.dma_start(
                        out=ag_in[qb, hp, h * DV : (h + 1) * DV], in_=ct[:]
                    )
                nc.gpsimd.collective_compute(
                    "AllGather",
                    mybir.AluOpType.bypass,
                    replica_groups=GROUPS,
                    ins=[ag_in[qb, hp]],
                    outs=[ag_out[qb, hp]],
                )

            # ---- gated output projection for this core's 256 columns ----
            def fc_block(qb):
                ot_all = fcin.tile([128, DCH, QB], F16, tag="fcin", name="ot_all")
                for hp in range(NHP):
                    nc.sync.dma_start(
                        out=ot_all[:, hp * NG : (hp + 1) * NG, :],
                        in_=ag_out[qb, hp].rearrange("(g p) q -> p g q", p=128),
                    )
                for t in range(2):  # two 128-column output tiles
                    fps = pp_fc.tile([128, QB], F32, tag="fcpsum", name="fps")
                    gps = pp_fc.tile([128, QB], F32, tag="fcpsum", name="gps")
                    for c in range(DCH):
                        nc.tensor.matmul(
                            fps[:],
                            lhsT=wfc_sb[:, c, t * 128 : (t + 1) * 128],
                            rhs=ot_all[:, c, :],
                            start=(c == 0),
                            stop=(c == DCH - 1),
                        )
                    for c in range(DCH):
                        nc.tensor.matmul(
                            gps[:],
                            lhsT=wg_sb[:, c, t * 128 : (t + 1) * 128],
                            rhs=ot_all[:, c, :],
                            start=(c == 0),
                            stop=(c == DCH - 1),
                        )
                    # sigmoid(g) = 0.5*tanh(g/2) + 0.5 — stays on the
                    # exp/tanh table set (no ~2.7us table reloads)
                    tanh_t = small.tile([128, QB], F16, tag="tanh")
                    sig_t = small.tile([128, QB], F16, tag="sig")
                    nc.scalar.activation(
                        out=tanh_t[:], in_=fps[:],
                        func=mybir.ActivationFunctionType.Tanh,
                    )
                    nc.scalar.activation(
                        out=sig_t[:], in_=gps[:],
                        func=mybir.ActivationFunctionType.Tanh, scale=0.5,
                    )
                    nc.vector.tensor_scalar(
                        out=sig_t[:],
                        in0=sig_t[:],
                        scalar1=0.5,
                        scalar2=0.5,
                        op0=mybir.AluOpType.mult,
                        op1=mybir.AluOpType.add,
                    )
                    res = small.tile([128, QB], F16, tag="res")
                    nc.vector.tensor_mul(out=res[:], in0=sig_t[:], in1=tanh_t[:])
                    nc.gpsimd.dma_start(
                        out=out[t * 128 : (t + 1) * 128, bass.ts(qb, QB)],
                        in_=res[:],
                    )

            # ---- emission order ----
            # Keys block 0 + q-block 0 first so attention starts early;
            # remaining K/V projections stream inside unit (0,0); each
            # unit's normalization is injected after the NEXT unit's first
            # S matmul (so the rb matmul never stalls the tensor FIFO);
            # fc blocks sit >=3 units after their gather was shipped to
            # tolerate cross-core launch skew.
            def kq(nt):
                return lambda: proj_kq(
                    kT, wk_sb,
                    [khTs[0][:, bass.ts(nt, 512)], khTs[1][:, bass.ts(nt, 512)]],
                    nt,
                )

            def qp(qb):
                return lambda: proj_kq(
                    qT, wq_sb, [qhTs[qb * NHP], qhTs[qb * NHP + 1]], qb
                )

            def vp(nt):
                return lambda: proj_v(nt)

            proj_kq(kT, wk_sb,
                    [khTs[0][:, bass.ts(0, 512)], khTs[1][:, bass.ts(0, 512)]], 0)
            nc.sync.dma_start(out=wv_sb[:], in_=wvT[:])
            proj_kq(qT, wq_sb, [qhTs[0], qhTs[1]], 0)

            ops = {}
            ops[(0, 0)] = attention(
                0, 0,
                pre_s={4: (kq(1), vp(1)), 8: (kq(2), vp(2)), 12: (kq(3), vp(3))},
                pre_o0=(vp(0),),
            )
            ops[(0, 1)] = attention(
                0, 1,
                pre_o0=(lambda: norm_ship(0, 0, ops[(0, 0)]), qp(1)),
            )
            nc.sync.dma_start(out=wfc_sb[:], in_=wfcT[:])
            ops[(1, 0)] = attention(
                1, 0, pre_o0=(lambda: norm_ship(0, 1, ops[(0, 1)]),)
            )
            ops[(1, 1)] = attention(
                1, 1,
                pre_o0=(lambda: norm_ship(1, 0, ops[(1, 0)]), qp(2)),
            )
            nc.sync.dma_start(out=wg_sb[:], in_=wgT[:])
            ops[(2, 0)] = attention(
                2, 0, pre_o0=(lambda: norm_ship(1, 1, ops[(1, 1)]),)
            )
            ops[(2, 1)] = attention(
                2, 1,
                pre_o0=(lambda: norm_ship(2, 0, ops[(2, 0)]), qp(3)),
            )
            fc_block(0)
            ops[(3, 0)] = attention(
                3, 0, pre_o0=(lambda: norm_ship(2, 1, ops[(2, 1)]),)
            )
            fc_block(1)
            ops[(3, 1)] = attention(
                3, 1, pre_o0=(lambda: norm_ship(3, 0, ops[(3, 0)]),)
            )
            norm_ship(3, 1, ops[(3, 1)])
            fc_block(2)
            fc_block(3)

    return nc


_NC_CACHE = None


def _get_nc():
    global _NC_CACHE
    if _NC_CACHE is None:
        _NC_CACHE = build_kernel()
    return _NC_CACHE


def prepare_inputs(q, k, v, Wq, bq, Wk, bk, Wv, bv, Wfc, bfc, Wg, bg):
    """Host-side layout prep: transpose + fp16 cast + per-core slices.

    Core c = (batch c//4, head-group c%4). Activations are pre-tiled to
    [128, nt, c, n] so each DMA partition line is one contiguous 8KB run.
    Biases are structurally zero in this problem and are folded out.
    """

    def tile_act(xb):
        # [L, D] -> [D, L] -> [c, p, nt, n] -> [p, nt, c, n]
        xT = np.ascontiguousarray(np.asarray(xb).reshape(L, D).T, np.float16)
        return np.ascontiguousarray(
            xT.reshape(DCH, 128, NQB, 512).transpose(1, 2, 0, 3)
        )

    def tile_w(wT):
        # [D, M] -> [c, p, M] -> [p, c, M]
        return np.ascontiguousarray(
            wT.reshape(DCH, 128, wT.shape[1]).transpose(1, 0, 2)
        )

    def tile_w_fc(wT_cols):
        # [H*DV, 256] -> chunks ordered c = hp*NG + gg, where chunk
        # (hp, gg) = rows [gg*256 + hp*128, +128) (rank gg's heads for hp)
        chunks = [
            wT_cols[gg * 256 + hp * 128 : gg * 256 + (hp + 1) * 128]
            for hp in range(NHP)
            for gg in range(NG)
        ]
        return np.ascontiguousarray(np.stack(chunks, 0).transpose(1, 0, 2))

    acts = [[tile_act(x[b]) for b in range(B)] for x in (q, k, v)]
    WqT = (np.asarray(Wq, np.float32) / TEMP).T.astype(np.float16)  # [D, H*DK]
    WkT = np.asarray(Wk, np.float32).T.astype(np.float16)
    WvT = np.asarray(Wv, np.float32).T.astype(np.float16)
    WfcT = np.asarray(Wfc, np.float32).T.astype(np.float16)  # [H*DV, D]
    WgT = np.asarray(Wg, np.float32).T.astype(np.float16)

    in_maps = []
    for c in range(NC):
        b, g = c // NG, c % NG
        hs = g * HC * DK  # 256-wide head slice
        in_maps.append(
            {
                "qT": acts[0][b],
                "kT": acts[1][b],
                "vT": acts[2][b],
                "wqT": tile_w(WqT[:, hs : hs + HC * DK]),
                "wkT": tile_w(WkT[:, hs : hs + HC * DK]),
                "wvT": tile_w(WvT[:, hs : hs + HC * DV]),
                "wfcT": tile_w_fc(WfcT[:, g * 256 : (g + 1) * 256]),
                "wgT": tile_w_fc(WgT[:, g * 256 : (g + 1) * 256]),
            }
        )
    return in_maps


def assemble_output(results):
    # core (b, g) produced output columns [g*256, (g+1)*256) of batch b,
    # transposed [256, L] fp16
    batches = []
    for b in range(B):
        cols = [np.asarray(results[b * NG + g]["out"]) for g in range(NG)]
        full = np.concatenate(cols, axis=0).astype(np.float32)  # [D, L]
        batches.append(np.ascontiguousarray(full.T))  # [L, D]
    return np.stack(batches, axis=0)


def kernel(**inputs):
    nc = _get_nc()
    in_maps = prepare_inputs(**{k: np.asarray(v) for k, v in inputs.items()})
    res = bass_utils.run_bass_kernel_spmd(nc, in_maps, core_ids=list(range(NC)))
    return assemble_output(res.results)


if __name__ == "__main__":
    nc = build_kernel()
    print("kernel built OK")
